# revision 11
# baseline (speedup 1.0000x reference)
"""AdaptiveTokenFilter Trainium2 kernel (8 NeuronCores, batch-parallel SPMD).

Per core (one batch row of B=8):
  pass 1: stream x [8192,1024] f32, compute logits = relu(x@W1+b1)@W2+b2
          via PE transposes (x tiles -> xT) + fp32r matmuls.
  select: expected_k = sum(sigmoid(logits)); k = floor;
          z = logits - ln(-ln(u)); find top-k threshold by 36-step
          binary expansion on the value axis (count(z>t) vs k);
          mask = z > lo_final (exactly k ones for tie-free rows).
  pass 2: filtered = x * mask[token]; 34/64 token tiles are kept in
          SBUF from pass 1, the rest are re-read.

Everything is hardcoded for the problem shapes:
  token_embeddings [8, 8192, 1024] f32, W1 [1024,64], b1 [64],
  W2 [64,1], b2 [1], u [8,8192].
"""

import numpy as np

import concourse.bass as bass
import concourse.mybir as mybir
from concourse import bacc
from concourse.bass import ts
from concourse.bass_utils import run_bass_kernel_spmd
from concourse.masks import make_identity
from concourse.tile import TileContext

F32 = mybir.dt.float32
F32R = mybir.dt.float32r
AF = mybir.ActivationFunctionType
OP = mybir.AluOpType

B, S, E, H = 8, 8192, 1024, 64
P = 128
NT = S // P          # 64 token tiles of 128 tokens
GT = 256             # tokens per pass-1 group
NSUB = GT // P       # 2 token tiles per group
NG = S // GT         # 32 groups
NE = E // P          # 8 e-chunks of 128
NSTASH = 34          # token tiles kept in SBUF between the passes
RR = 8               # re-read pool slots
NBIS = 36            # threshold search iterations
LO0, W0 = -8.0, 32.0  # z search range [-8, 24)

_CACHE = {}


def build_nc(mm_f32r=True, transpose_f32r=False):
    nc = bacc.Bacc("TRN2", target_bir_lowering=False, debug=False, num_devices=B)
    x_ext = nc.dram_tensor("x", [S, E], F32, kind="ExternalInput").ap()
    w1_ext = nc.dram_tensor("w1", [E, H], F32, kind="ExternalInput").ap()
    b1_ext = nc.dram_tensor("b1", [H], F32, kind="ExternalInput").ap()
    w2_ext = nc.dram_tensor("w2", [H, 1], F32, kind="ExternalInput").ap()
    b2_ext = nc.dram_tensor("b2", [1], F32, kind="ExternalInput").ap()
    u_ext = nc.dram_tensor("u", [S], F32, kind="ExternalInput").ap()
    out_ext = nc.dram_tensor("out", [S, E], F32, kind="ExternalOutput").ap()
    mask_ext = nc.dram_tensor("mask", [S], F32, kind="ExternalOutput").ap()
    ek_ext = nc.dram_tensor("ek", [1], F32, kind="ExternalOutput").ap()

    mm_dt = F32R if mm_f32r else F32
    tr_dt = F32R if transpose_f32r else F32

    with TileContext(nc) as tc, \
            tc.tile_pool(name="const", bufs=1) as cpool, \
            tc.tile_pool(name="stash", bufs=1) as stash_pool, \
            tc.tile_pool(name="rr", bufs=RR) as rr_pool, \
            tc.tile_pool(name="xt", bufs=4) as xt_pool, \
            tc.tile_pool(name="hT", bufs=2) as ht_pool, \
            tc.tile_pool(name="small", bufs=1) as spool:

        # ---------------- constants ----------------
        w1_raw = cpool.tile([P, NE, H], F32)
        nc.sync.dma_start(out=w1_raw[:], in_=w1_ext.rearrange("(c p) h -> p c h", p=P))
        w1_sb = cpool.tile([P, NE, H], F32)
        if mm_dt == F32R:
            nc.vector.tensor_copy(w1_sb[:].bitcast(F32R), w1_raw[:])
        else:
            w1_sb = w1_raw
        b1_sb = cpool.tile([H, 1], F32)
        nc.sync.dma_start(out=b1_sb[:], in_=b1_ext.rearrange("(h one) -> h one", one=1))
        w2_sb = cpool.tile([H, 1], F32)
        nc.sync.dma_start(out=w2_sb[:], in_=w2_ext)
        b2_sb = cpool.tile([1, 1], F32)
        nc.sync.dma_start(out=b2_sb[:], in_=b2_ext.rearrange("(o one) -> o one", one=1))
        u_sb = cpool.tile([P, NT], F32)
        nc.sync.dma_start(out=u_sb[:], in_=u_ext.rearrange("(j p) -> p j", p=P))
        ones_sb = cpool.tile([P, P], F32)
        nc.vector.memset(ones_sb[:], 1.0)
        ident_sb = cpool.tile([P, P], F32)
        make_identity(nc, ident_sb[:])
        logits_sb = cpool.tile([P, NT], F32)

        stash_tiles = {}

        # ---------------- pass 1: logits ----------------
        with tc.tile_pool(name="xt_ps", bufs=4, space="PSUM") as xtps, \
                tc.tile_pool(name="h_ps", bufs=2, space="PSUM") as hps, \
                tc.tile_pool(name="lg_ps", bufs=2, space="PSUM") as lgps:

            # broadcast b2 scalar to all 128 partitions
            b2_ps = lgps.tile([P, 1], F32, tag="lg_ps")
            nc.tensor.matmul(b2_ps[:], ones_sb[0:1, :], b2_sb[:], start=True, stop=True)
            b2b_sb = spool.tile([P, 1], F32)
            nc.scalar.copy(b2b_sb[:], b2_ps[:])

            for g in range(NG):
                xins = []
                for s2 in range(NSUB):
                    i = g * NSUB + s2
                    if i >= NT - NSTASH:
                        xin = stash_pool.tile([P, E], F32, tag=f"stash{i}")
                        stash_tiles[i] = xin
                    else:
                        xin = rr_pool.tile([P, E], F32, tag="rr")
                    nc.sync.dma_start(out=xin[:], in_=x_ext[ts(i, P), :])
                    xins.append(xin)

                h_ps = hps.tile([H, GT], F32)
                for c in range(NE):
                    xt_ps = xtps.tile([P, GT], F32)
                    for s2 in range(NSUB):
                        nc.tensor.transpose(
                            xt_ps[:, ts(s2, P)].bitcast(tr_dt),
                            xins[s2][:, ts(c, P)].bitcast(tr_dt),
                            ident_sb[:].bitcast(tr_dt),
                        )
                    xt_sb = xt_pool.tile([P, GT], F32)
                    xt_out = xt_sb[:].bitcast(F32R) if mm_dt == F32R else xt_sb[:]
                    if c % 2 == 0:
                        nc.vector.tensor_copy(xt_out, xt_ps[:])
                    else:
                        nc.scalar.copy(xt_out, xt_ps[:])
                    nc.tensor.matmul(
                        h_ps[:],
                        w1_sb[:, c, :].bitcast(mm_dt),
                        xt_sb[:].bitcast(mm_dt),
                        start=(c == 0),
                        stop=(c == NE - 1),
                    )

                relu_sb = ht_pool.tile([H, GT], F32)
                nc.scalar.activation(relu_sb[:], h_ps[:], AF.Relu, bias=b1_sb[:])

                lg_ps = lgps.tile([P, NSUB], F32, tag="lg_ps")
                for s2 in range(NSUB):
                    nc.tensor.matmul(
                        lg_ps[:, s2:s2 + 1], relu_sb[:, ts(s2, P)], w2_sb[:],
                        start=True, stop=True,
                    )
                nc.scalar.activation(
                    logits_sb[:, ts(g, NSUB)], lg_ps[:], AF.Identity, bias=b2b_sb[:]
                )

        # ------------- early re-read DMAs (prefetch during selection) -------------
        rr_tiles = {}
        for i in range(NT - NSTASH):
            xin = rr_pool.tile([P, E], F32, tag="rr")
            nc.sync.dma_start(out=xin[:], in_=x_ext[ts(i, P), :])
            rr_tiles[i] = xin

        # ---------------- selection ----------------
        with tc.tile_pool(name="sel_ps", bufs=2, space="PSUM") as selps:
            # z = logits - ln(-ln(u))
            l1 = spool.tile([P, NT], F32)
            nc.scalar.activation(l1[:], u_sb[:], AF.Ln)
            l2 = spool.tile([P, NT], F32)
            nc.scalar.activation(l2[:], l1[:], AF.Ln, scale=-1.0)
            z_sb = spool.tile([P, NT], F32)
            nc.vector.tensor_sub(z_sb[:], logits_sb[:], l2[:])

            # expected_k = sum(sigmoid(logits)); km1 = k - 1
            sg = spool.tile([P, NT], F32)
            sgp = spool.tile([P, 1], F32)
            nc.scalar.activation(sg[:], logits_sb[:], AF.Sigmoid, accum_out=sgp[:])
            k_ps = selps.tile([P, 1], F32)
            nc.tensor.matmul(k_ps[:], ones_sb[:], sgp[:], start=True, stop=True)
            k_sb = spool.tile([P, 1], F32)
            nc.scalar.copy(k_sb[:], k_ps[:])
            km1_sb = spool.tile([P, 1], F32)
            nc.vector.tensor_scalar(km1_sb[:], k_ps[:], -1.0, None, op0=OP.add)
            nc.sync.dma_start(out=ek_ext, in_=k_sb[0:1, :])

            # binary expansion of the largest grid point lo with count(z>lo) >= k
            lo_sb = spool.tile([P, 1], F32)
            nc.vector.memset(lo_sb[:], LO0)
            t_sb = spool.tile([P, 1], F32)
            part_sb = spool.tile([P, 1], F32)
            pred_sb = spool.tile([P, 1], mybir.dt.uint32)
            cmp_sb = spool.tile([P, NT], F32)
            w = W0
            for _ in range(NBIS):
                w *= 0.5
                nc.vector.tensor_scalar(t_sb[:], lo_sb[:], w, None, op0=OP.add)
                nc.vector.tensor_scalar(
                    cmp_sb[:], z_sb[:], t_sb[:], None, op0=OP.is_gt, op1=OP.add,
                    accum_out=part_sb[:],
                )
                cnt_ps = selps.tile([P, 1], F32)
                nc.tensor.matmul(cnt_ps[:], ones_sb[:], part_sb[:], start=True, stop=True)
                nc.vector.tensor_tensor(pred_sb[:], cnt_ps[:], km1_sb[:], op=OP.is_gt)
                nc.vector.copy_predicated(lo_sb[:], pred_sb[:], t_sb[:])

            mask_sb = spool.tile([P, NT], F32)
            nc.vector.tensor_scalar(mask_sb[:], z_sb[:], lo_sb[:], None, op0=OP.is_gt)
            nc.sync.dma_start(
                out=mask_ext.rearrange("(j p) -> p j", p=P), in_=mask_sb[:]
            )

        # ---------------- pass 2: filtered = x * mask ----------------
        # stashed tiles first: ready the moment the mask lands
        for n, i in enumerate(range(NT - NSTASH, NT)):
            xin = stash_tiles[i]
            col = mask_sb[:, i:i + 1]
            if n % 2 == 0:
                nc.vector.tensor_scalar_mul(xin[:], xin[:], col)
            else:
                nc.scalar.mul(xin[:], xin[:], col)
            nc.sync.dma_start(out=out_ext[ts(i, P), :], in_=xin[:])
        for i in range(NT - NSTASH):
            xin = rr_tiles[i]
            col = mask_sb[:, i:i + 1]
            if i % 2 == 0:
                nc.vector.tensor_scalar_mul(xin[:], xin[:], col)
            else:
                nc.scalar.mul(xin[:], xin[:], col)
            nc.sync.dma_start(out=out_ext[ts(i, P), :], in_=xin[:])

    nc.compile()
    return nc


def _get_nc():
    if "nc" not in _CACHE:
        _CACHE["nc"] = build_nc()
    return _CACHE["nc"]


def run(inputs, trace=False, trace_cores=None):
    """Run the SPMD kernel on all 8 cores. Returns (outputs, results_obj)."""
    nc = _get_nc()
    x = np.ascontiguousarray(np.asarray(inputs["token_embeddings"], dtype=np.float32))
    u = np.ascontiguousarray(np.asarray(inputs["u"], dtype=np.float32))
    w1 = np.ascontiguousarray(np.asarray(inputs["W1"], dtype=np.float32))
    b1 = np.ascontiguousarray(np.asarray(inputs["b1"], dtype=np.float32))
    w2 = np.ascontiguousarray(np.asarray(inputs["W2"], dtype=np.float32))
    b2 = np.ascontiguousarray(np.asarray(inputs["b2"], dtype=np.float32))

    in_maps = [
        {"x": x[i], "w1": w1, "b1": b1, "w2": w2, "b2": b2, "u": u[i]}
        for i in range(B)
    ]
    res = run_bass_kernel_spmd(
        nc, in_maps, list(range(B)), trace=trace, trace_cores=trace_cores
    )
    filtered = np.stack([res.results[i]["out"] for i in range(B)])
    mask = np.stack([res.results[i]["mask"] for i in range(B)])
    ek = np.concatenate([res.results[i]["ek"] for i in range(B)])
    return (filtered, mask, ek), res


def kernel(token_embeddings, W1, b1, W2, b2, u):
    outs, _ = run(
        {
            "token_embeddings": token_embeddings,
            "W1": W1,
            "b1": b1,
            "W2": W2,
            "b2": b2,
            "u": u,
        }
    )
    return outs
